# revision 23
# baseline (speedup 1.0000x reference)
"""Trainium2 Bass kernel for nn_CorrBlockSingleScale (RAFT single-scale
correlation lookup), distributed over 8 NeuronCores.

  fmap1, fmap2: [1, 256, 64, 96] f32;  coords: [1, 2, 64, 96] f32; radius=4
  corr = einsum('bcm,bcn->bmn', f1, f2) / 16        -> [6144, 64, 96]
  out[q, i, j] = bilinear(corr[q], (cx_q + d_i, cy_q + d_j)),  d in -4..4
  output [1, 81, 64, 96] f32.

Structure exploited: the 9x9 sample offsets are integers, so all 81 samples
of a query share one fractional pair (fx, fy) -- the output is a separable
2x2-tap blend of a 10x10 patch of corr[q] anchored at
(floor(cx)-4, floor(cy)-4).

Each query only reads a 10x10 patch of its 64x96 corr plane, so queries are
k-d clustered on the host by their (coord) positions into 48 clusters of
exactly 128; a cluster's union of patches is a small (PX x PY) rectangle
(~22x22 = ~490 elements) instead of a full y-band.  Zero-padding the
per-cluster f2 slabs (both to the patch rectangle and up to a 512-column
PSUM bank) makes out-of-image taps exact zeros -- no validity masks -- and
gives every cluster an identical single-bank matmul.

DMA-instruction count dominates this kernel (each dma_start costs ~0.6us of
sequencer/descriptor-generation time), so transfers are batched per core:
  - 3 packed byte-tensor input DMAs, one per cluster pair (f1 slice + two
    512-padded f2 slabs; the first also carries gather indices + weights)
  - per pair: 4 bank-aligned matmuls into one 2-bank PSUM tile, ONE
    PSUM->SBUF fp16 convert-copy, ONE fp16 scratch write, ONE
    256-descriptor indirect gather (contiguous 9*PY+10 window per query)
  - 4-op separable bilinear blend per cluster (DVE) into a staged output
    tile; ONE packed output DMA
issued from different engines (SP / ACT / Pool / DVE) to overlap dispatch.
For the timing loop the body is instantiated twice per For_i iteration
with alternating tile-pool buffers and scratch tensors, so consecutive
iterations pipeline across the loop back-edge.
Host post-pass inverse-permutes to the reference layout.
"""


import numpy as np

import concourse.bass as bass
import concourse.bacc as bacc
import concourse.mybir as mybir
import concourse.tile as tile
from concourse import bass_utils

F32 = mybir.dt.float32
F16 = mybir.dt.float16
BF16 = mybir.dt.bfloat16
I32 = mybir.dt.int32
U8 = mybir.dt.uint8

B, C, H, W = 1, 256, 64, 96
R = 4
K = 2 * R + 1          # 9
PK = K + 1             # 10 (patch side)
NQ = H * W             # 6144
NCORES = 8
P = 128
NT = 6                 # clusters (tiles) per core
NG = 3                 # scratch/gather groups of 2 clusters
BANK = 512             # PSUM bank (f32 elements); patch slabs pad to this


# --------------------------------------------------------------------------
# host-side preprocessing
# --------------------------------------------------------------------------

def _kd_split(idx, key, n):
    """Split index array into n equal-count chunks by rank of key."""
    o = idx[np.argsort(key[idx], kind="stable")]
    m = len(idx) // n
    return [o[i * m:(i + 1) * m] for i in range(n)]


def _cluster(cx, cy):
    """48 clusters of exactly 128 queries, clustered on (cx, cy)."""
    schemes = [
        [("x", 8), ("y", 6)],
        [("x", 2), ("y", 2), ("x", 4), ("y", 3)],
        [("x", 2), ("y", 3), ("x", 4), ("y", 2)],
        [("y", 6), ("x", 8)],
        [("x", 6), ("y", 8)],
        [("y", 8), ("x", 6)],
        [("x", 2), ("y", 4), ("x", 3), ("y", 2)],
        [("y", 2), ("x", 4), ("y", 3), ("x", 2)],
        [("y", 4), ("x", 12)],
        [("y", 3), ("x", 4), ("y", 2), ("x", 2)],
        [("x", 4), ("y", 4), ("x", 3)],
    ]
    jx = np.floor(cx)
    jy = np.floor(cy)
    best = None
    for sch in schemes:
        groups = [np.arange(NQ)]
        for ax, n in sch:
            key = cx if ax == "x" else cy
            groups = [g for grp in groups for g in _kd_split(grp, key, n)]
        # slot assignment: sort by patch area desc, slot t <- ranks [8t, 8t+8)
        areas = []
        dims = []
        for g in groups:
            px = int(jx[g].max() - jx[g].min()) + PK
            py = int(jy[g].max() - jy[g].min()) + PK
            areas.append(px * py)
            dims.append((px, py))
        srt = np.argsort(-np.asarray(areas), kind="stable")
        cost = 0
        for t in range(NT):
            slot = srt[t * NCORES:(t + 1) * NCORES]
            pxm = max(dims[i][0] for i in slot)
            pym = max(dims[i][1] for i in slot)
            # patches beyond one PSUM bank force extra chunks: avoid hard
            cost += pxm * pym + (1_000_000 if pxm * pym > BANK else 0)
        if best is None or cost < best[0]:
            best = (cost, groups, srt)
    _, groups, srt = best
    clusters = [[None] * NT for _ in range(NCORES)]
    for t in range(NT):
        slot = srt[t * NCORES:(t + 1) * NCORES]
        for c in range(NCORES):
            clusters[c][t] = groups[slot[c]]
    return clusters


def host_preprocess(fmap1, fmap2, coords):
    """Returns (in_maps, order, shapes)."""
    import ml_dtypes
    bf16 = ml_dtypes.bfloat16
    f1 = np.asarray(fmap1, np.float32).reshape(C, NQ)
    f2 = np.asarray(fmap2, np.float32).reshape(C, H, W)
    cx = np.asarray(coords, np.float32)[0, 0].reshape(NQ)
    cy = np.asarray(coords, np.float32)[0, 1].reshape(NQ)

    ix = np.floor(cx)
    iy = np.floor(cy)
    fx = (cx - ix).astype(np.float32)   # exact in fp32
    fy = (cy - iy).astype(np.float32)
    jx = ix.astype(np.int64)
    jy = iy.astype(np.int64)

    clusters = _cluster(cx, cy)

    # uniform per-slot patch shapes across cores
    shapes = []
    for t in range(NT):
        pxm = max(int(jx[clusters[c][t]].max() - jx[clusters[c][t]].min())
                  + PK for c in range(NCORES))
        pym = max(int(jy[clusters[c][t]].max() - jy[clusters[c][t]].min())
                  + PK for c in range(NCORES))
        shapes.append((pxm, pym))
    # order slots by py ascending so that within each gather pair the
    # second tile has the larger window: the merged gather then never reads
    # past the written scratch region (window overruns stay inside data)
    perm = sorted(range(NT), key=lambda t: shapes[t][1])
    shapes = tuple(shapes[t] for t in perm)
    for c in range(NCORES):
        clusters[c] = [clusters[c][t] for t in perm]
    pads = [BANK * ((px * py + BANK - 1) // BANK) for px, py in shapes]

    in_maps = []
    order = np.empty(NQ, np.int64)
    pos = 0
    for c in range(NCORES):
        m = {}
        qorder = np.concatenate([clusters[c][t] for t in range(NT)])
        order[pos:pos + NT * P] = qorder
        pos += NT * P

        idx = np.empty((P, NT), np.int32)
        wts = np.stack([(1.0 - fy[qorder]), fy[qorder],
                        (1.0 - fx[qorder]) / 16.0, fx[qorder] / 16.0],
                       axis=1).astype(np.float32)
        wtsP = wts.reshape(NT, P, 4).transpose(1, 0, 2).reshape(P, NT * 4)

        slabs = []
        for t in range(NT):
            qs = clusters[c][t]
            px, py = shapes[t]
            g, j = divmod(t, 2)
            x0 = int(jx[qs].min()) - R
            y0 = int(jy[qs].min()) - R
            # zero-padded [C, pads[t]] patch slab (x-major, y minor)
            slab = np.zeros((C, pads[t]), np.float32)
            sl = slab[:, :px * py].reshape(C, px, py)
            xs0, xs1 = max(x0, 0), min(x0 + px, W)
            ys0, ys1 = max(y0, 0), min(y0 + py, H)
            if xs1 > xs0 and ys1 > ys0:
                sl[:, xs0 - x0:xs1 - x0, ys0 - y0:ys1 - y0] = \
                    f2[:, ys0:ys1, xs0:xs1].transpose(0, 2, 1)
            sb = slab.reshape(2, P, pads[t]).astype(bf16)
            slabs.append(np.ascontiguousarray(
                np.concatenate([sb[0], sb[1]], axis=1)).view(np.uint8))
            sg = pads[2 * g] + pads[2 * g + 1]
            base = pads[2 * g] if j else 0
            rel = (jx[qs] - R - x0) * py + (jy[qs] - R - y0)
            idx[:, t] = (np.arange(P) * sg + base + rel).astype(np.int32)

        f1b = f1[:, qorder].reshape(2, P, NT * P).astype(bf16)
        parts = [idx.view(np.uint8), np.ascontiguousarray(wtsP).view(np.uint8)]
        for g in range(NG):
            f1g = np.ascontiguousarray(np.concatenate(
                [f1b[0, :, g * 256:(g + 1) * 256],
                 f1b[1, :, g * 256:(g + 1) * 256]], axis=1)).view(np.uint8)
            parts += [f1g, slabs[2 * g], slabs[2 * g + 1]]
        m["in0"] = np.ascontiguousarray(np.concatenate(parts, axis=1))
        in_maps.append(m)
    return in_maps, order, shapes


def assemble_output(results, order):
    # device emits [P, NT*81] partition-major; restore (tile, p) query order
    rows = np.concatenate(
        [results[c]["out"].reshape(P, NT, K * K).transpose(1, 0, 2)
         .reshape(NT * P, K * K) for c in range(NCORES)],
        axis=0)
    # device blend emits [dx, dy]-major, matching the reference's 81-axis
    # (delta[..., 0] is added to x and varies along the first grid axis)
    full = np.empty((K * K, NQ), np.float32)
    full[:, order] = rows.T.astype(np.float32)
    return full.reshape(1, K * K, H, W)


# --------------------------------------------------------------------------
# device program
# --------------------------------------------------------------------------

IDX_BYTES = NT * 4          # [P, NT] i32
WTS_BYTES = NT * 4 * 4      # [P, NT*4] f32
F1G_BYTES = 2 * 256 * 2     # [P, 2*256] bf16 per group


def _body(tc, nc, aps, scr, shapes, pools, parity=0):
    const, corr_pool, psum_pool, small = pools
    e0, e1 = (nc.sync, nc.scalar) if parity == 0 else (nc.scalar, nc.sync)
    pads = [BANK * ((px * py + BANK - 1) // BANK) for px, py in shapes]
    sg_sizes = [pads[2 * g] + pads[2 * g + 1] for g in range(NG)]
    wins = [(PK - 1) * py + PK for _, py in shapes]
    wmaxs = [max(wins[2 * g], wins[2 * g + 1]) for g in range(NG)]

    hdr0 = IDX_BYTES + WTS_BYTES
    goff = [hdr0]
    for g in range(NG):
        goff.append(goff[-1] + F1G_BYTES
                    + 4 * (pads[2 * g] + pads[2 * g + 1]))
    pack = const.tile([P, goff[-1]], U8, tag="pack")
    e0.dma_start(pack[:], aps["in0"])

    idxb = pack[:, 0:IDX_BYTES].bitcast(I32)                       # [P, NT]
    wtsb = pack[:, IDX_BYTES:IDX_BYTES + WTS_BYTES].bitcast(F32)

    otb = const.tile([P, NT * K * K], F32, tag="otb")

    for g in range(NG):
        f1g = pack[:, goff[g]:goff[g] + F1G_BYTES].bitcast(BF16)   # [P, 512]
        sg = sg_sizes[g]
        wmax = wmaxs[g]

        ps = psum_pool.tile([P, sg], F32, space="PSUM", tag="ps")
        for j in range(2):
            t = 2 * g + j
            pad = pads[t]
            off = pads[2 * g] if j else 0
            f2v = pack[:, goff[g] + F1G_BYTES + off * 4:
                       goff[g] + F1G_BYTES + (off + pad) * 4].bitcast(BF16)
            for ci in range(pad // BANK):
                for k in range(2):
                    lhsT = f1g[:, k * 256 + j * P: k * 256 + (j + 1) * P]
                    rhs = f2v[:, k * pad + ci * BANK:
                              k * pad + ci * BANK + BANK]
                    nc.tensor.matmul(
                        ps[:, off + ci * BANK: off + (ci + 1) * BANK],
                        lhsT=lhsT, rhs=rhs, start=(k == 0), stop=(k == 1))

        corr_g = corr_pool.tile([P, max(sg_sizes)], F16, tag="corr")
        nc.vector.tensor_copy(corr_g[:, 0:sg], ps[:])

        dst = scr[g].ap()[0:P * sg].rearrange("(p f) -> p f", p=P)
        [e1, e0, e1][g].dma_start(dst, corr_g[:, 0:sg])

        src = scr[g].ap().rearrange("(n o) -> n o", o=1)
        pts = []
        for j in range(2):
            t = 2 * g + j
            # multi-offset indirect DMA is broken on HW: one gather per tile
            pt = small.tile([P, PK * shapes[t][1]], F16, tag=f"pt{j}")
            nc.gpsimd.indirect_dma_start(
                out=pt[:, 0:wins[t]], out_offset=None, in_=src,
                in_offset=bass.IndirectOffsetOnAxis(
                    ap=idxb[:, t:t + 1], axis=0))
            pts.append(pt)

        for j in range(2):
            t = 2 * g + j
            py = shapes[t][1]
            ptv = pts[j][:].rearrange("p (b r) -> p b r", r=py)[:, :, 0:PK]

            t1 = small.tile([P, PK * K], F16, tag="t1")
            t13 = t1[:].rearrange("p (a b) -> p a b", b=K)
            nc.vector.tensor_scalar_mul(
                t13, ptv[:, :, 1:PK], wtsb[:, 4 * t + 1: 4 * t + 2])
            cm = small.tile([P, PK * K], F16, tag="cm")
            cm3 = cm[:].rearrange("p (a b) -> p a b", b=K)
            nc.vector.scalar_tensor_tensor(
                cm3, ptv[:, :, 0:K], wtsb[:, 4 * t: 4 * t + 1], t13,
                op0=mybir.AluOpType.mult, op1=mybir.AluOpType.add)

            t2 = small.tile([P, K * K], F16, tag="t2")
            t23 = t2[:].rearrange("p (a b) -> p a b", b=K)
            nc.vector.tensor_scalar_mul(
                t23, cm3[:, 1:PK, :], wtsb[:, 4 * t + 3: 4 * t + 4])
            ot3 = otb[:, t * K * K:(t + 1) * K * K] \
                .rearrange("p (a b) -> p a b", b=K)
            nc.vector.scalar_tensor_tensor(
                ot3, cm3[:, 0:K, :], wtsb[:, 4 * t + 2: 4 * t + 3], t23,
                op0=mybir.AluOpType.mult, op1=mybir.AluOpType.add)

    # out is partition-major [P, NT*81]; the host transposes to query order
    e0.dma_start(aps["out"], otb[:])


def build_program(shapes, rep=1):
    """rep>1 wraps a double body in a For_i(rep//2) loop (for timing)."""
    nc = bacc.Bacc("TRN2", target_bir_lowering=False, debug=False,
                   num_devices=NCORES)
    pads = [BANK * ((px * py + BANK - 1) // BANK) for px, py in shapes]
    aps = {}
    nbytes = IDX_BYTES + WTS_BYTES + sum(
        F1G_BYTES + 4 * (pads[2 * g] + pads[2 * g + 1]) for g in range(NG))
    aps["in0"] = nc.dram_tensor("in0", [P, nbytes], U8,
                                kind="ExternalInput").ap()
    aps["out"] = nc.dram_tensor("out", [P, NT * K * K], F32,
                                kind="ExternalOutput").ap()
    psum_bufs = max(1, min(4, 8 * BANK // max(
        pads[2 * g] + pads[2 * g + 1] for g in range(NG))))
    unroll = min(2, rep)
    scr = [[nc.dram_tensor(f"scr{b}_{g}",
                           [P * (pads[2 * g] + pads[2 * g + 1])], F16)
            for g in range(NG)] for b in range(unroll)]

    with tile.TileContext(nc) as tc:
        def mk_pools(ctx):
            return (ctx.enter_context(tc.tile_pool(name="const", bufs=3)),
                    ctx.enter_context(tc.tile_pool(name="corr", bufs=3)),
                    ctx.enter_context(
                        tc.tile_pool(name="ps", bufs=psum_bufs,
                                     space="PSUM")),
                    ctx.enter_context(tc.tile_pool(name="small", bufs=3)))

        import contextlib
        with contextlib.ExitStack() as ctx:
            pools = mk_pools(ctx)
            if rep == 1:
                _body(tc, nc, aps, scr[0], shapes, pools)
            else:
                tail = rep % unroll
                with tc.For_i(0, rep // unroll, staggered_reset=True):
                    for b in range(unroll):
                        _body(tc, nc, aps, scr[b], shapes, pools, b % 2)
                for b in range(tail):
                    _body(tc, nc, aps, scr[b], shapes, pools, b % 2)
    nc.compile()
    return nc


_PROGRAMS = {}


def kernel(fmap1, fmap2, coords, radius):
    assert int(radius) == R, f"kernel hardcodes radius=4, got {radius}"
    in_maps, order, shapes = host_preprocess(fmap1, fmap2, coords)
    nc = _PROGRAMS.get(shapes)
    if nc is None:
        nc = _PROGRAMS[shapes] = build_program(shapes)
    last_err = None
    for _ in range(3):  # the remote compile hook occasionally flakes
        try:
            res = bass_utils.run_bass_kernel_spmd(
                nc, in_maps, core_ids=list(range(NCORES)))
            return assemble_output(res.results, order)
        except Exception as e:  # noqa: BLE001
            last_err = e
    raise last_err


# revision 27
# speedup vs baseline: 1.1438x; 1.1438x over previous
"""Trainium2 Bass kernel for nn_CorrBlockSingleScale (RAFT single-scale
correlation lookup), distributed over 8 NeuronCores.

  fmap1, fmap2: [1, 256, 64, 96] f32;  coords: [1, 2, 64, 96] f32; radius=4
  corr = einsum('bcm,bcn->bmn', f1, f2) / 16        -> [6144, 64, 96]
  out[q, i, j] = bilinear(corr[q], (cx_q + d_i, cy_q + d_j)),  d in -4..4
  output [1, 81, 64, 96] f32.

Structure exploited: the 9x9 sample offsets are integers, so all 81 samples
of a query share one fractional pair (fx, fy) -- the output is a separable
2x2-tap blend of a 10x10 patch of corr[q] anchored at
(floor(cx)-4, floor(cy)-4).

Each query only reads a 10x10 patch of its 64x96 corr plane, so queries are
k-d clustered on the host by their (coord) positions into 48 clusters of
exactly 128; a cluster's union of patches is a small (PX x PY) rectangle
(~22x22 = ~490 elements) instead of a full y-band.  Zero-padding the
per-cluster f2 slabs (both to the patch rectangle and up to a 512-column
PSUM bank) makes out-of-image taps exact zeros -- no validity masks -- and
gives every cluster an identical single-bank matmul.

DMA-instruction count dominates this kernel (each dma_start costs ~0.6us of
sequencer/descriptor-generation time), so transfers are batched per core:
  - 3 packed byte-tensor input DMAs, one per cluster pair (f1 slice + two
    512-padded f2 slabs; the first also carries gather indices + weights)
  - per pair: 4 bank-aligned matmuls into one 2-bank PSUM tile, ONE
    PSUM->SBUF fp16 convert-copy, ONE fp16 scratch write, ONE
    256-descriptor indirect gather (contiguous 9*PY+10 window per query)
  - 4-op separable bilinear blend per cluster (DVE) into a staged output
    tile; ONE packed output DMA
issued from different engines (SP / ACT / Pool / DVE) to overlap dispatch.
For the timing loop the body is instantiated twice per For_i iteration
with alternating tile-pool buffers and scratch tensors, so consecutive
iterations pipeline across the loop back-edge.
Host post-pass inverse-permutes to the reference layout.
"""


import numpy as np

import concourse.bass as bass
import concourse.bacc as bacc
import concourse.mybir as mybir
import concourse.tile as tile
from concourse import bass_utils

F32 = mybir.dt.float32
F16 = mybir.dt.float16
BF16 = mybir.dt.bfloat16
I32 = mybir.dt.int32
U8 = mybir.dt.uint8

B, C, H, W = 1, 256, 64, 96
R = 4
K = 2 * R + 1          # 9
PK = K + 1             # 10 (patch side)
NQ = H * W             # 6144
NCORES = 8
P = 128
NT = 6                 # clusters (tiles) per core
NG = 3                 # scratch/gather groups of 2 clusters
BANK = 512             # PSUM bank (f32 elements); patch slabs pad to this


# --------------------------------------------------------------------------
# host-side preprocessing
# --------------------------------------------------------------------------

def _kd_split(idx, key, n):
    """Split index array into n equal-count chunks by rank of key."""
    o = idx[np.argsort(key[idx], kind="stable")]
    m = len(idx) // n
    return [o[i * m:(i + 1) * m] for i in range(n)]


def _cluster(cx, cy):
    """48 clusters of exactly 128 queries, clustered on (cx, cy)."""
    schemes = [
        [("x", 8), ("y", 6)],
        [("x", 2), ("y", 2), ("x", 4), ("y", 3)],
        [("x", 2), ("y", 3), ("x", 4), ("y", 2)],
        [("y", 6), ("x", 8)],
        [("x", 6), ("y", 8)],
        [("y", 8), ("x", 6)],
        [("x", 2), ("y", 4), ("x", 3), ("y", 2)],
        [("y", 2), ("x", 4), ("y", 3), ("x", 2)],
        [("y", 4), ("x", 12)],
        [("y", 3), ("x", 4), ("y", 2), ("x", 2)],
        [("x", 4), ("y", 4), ("x", 3)],
    ]
    jx = np.floor(cx)
    jy = np.floor(cy)
    best = None
    for sch in schemes:
        groups = [np.arange(NQ)]
        for ax, n in sch:
            key = cx if ax == "x" else cy
            groups = [g for grp in groups for g in _kd_split(grp, key, n)]
        # slot assignment: sort by patch area desc, slot t <- ranks [8t, 8t+8)
        areas = []
        dims = []
        for g in groups:
            px = int(jx[g].max() - jx[g].min()) + PK
            py = int(jy[g].max() - jy[g].min()) + PK
            areas.append(px * py)
            dims.append((px, py))
        srt = np.argsort(-np.asarray(areas), kind="stable")
        cost = 0
        for t in range(NT):
            slot = srt[t * NCORES:(t + 1) * NCORES]
            pxm = max(dims[i][0] for i in slot)
            pym = max(dims[i][1] for i in slot)
            # patches beyond one PSUM bank force extra chunks: avoid hard
            cost += pxm * pym + (1_000_000 if pxm * pym > BANK else 0)
        if best is None or cost < best[0]:
            best = (cost, groups, srt)
    _, groups, srt = best
    clusters = [[None] * NT for _ in range(NCORES)]
    for t in range(NT):
        slot = srt[t * NCORES:(t + 1) * NCORES]
        for c in range(NCORES):
            clusters[c][t] = groups[slot[c]]
    return clusters


def host_preprocess(fmap1, fmap2, coords):
    """Returns (in_maps, order, shapes)."""
    import ml_dtypes
    bf16 = ml_dtypes.bfloat16
    f1 = np.asarray(fmap1, np.float32).reshape(C, NQ)
    f2 = np.asarray(fmap2, np.float32).reshape(C, H, W)
    cx = np.asarray(coords, np.float32)[0, 0].reshape(NQ)
    cy = np.asarray(coords, np.float32)[0, 1].reshape(NQ)

    ix = np.floor(cx)
    iy = np.floor(cy)
    fx = (cx - ix).astype(np.float32)   # exact in fp32
    fy = (cy - iy).astype(np.float32)
    jx = ix.astype(np.int64)
    jy = iy.astype(np.int64)

    clusters = _cluster(cx, cy)

    # uniform per-slot patch shapes across cores
    shapes = []
    for t in range(NT):
        pxm = max(int(jx[clusters[c][t]].max() - jx[clusters[c][t]].min())
                  + PK for c in range(NCORES))
        pym = max(int(jy[clusters[c][t]].max() - jy[clusters[c][t]].min())
                  + PK for c in range(NCORES))
        shapes.append((pxm, pym))
    # order slots by py ascending so that within each gather pair the
    # second tile has the larger window: the merged gather then never reads
    # past the written scratch region (window overruns stay inside data)
    perm = sorted(range(NT), key=lambda t: shapes[t][1])
    shapes = tuple(shapes[t] for t in perm)
    for c in range(NCORES):
        clusters[c] = [clusters[c][t] for t in perm]
    pads = [BANK * ((px * py + BANK - 1) // BANK) for px, py in shapes]

    in_maps = []
    order = np.empty(NQ, np.int64)
    pos = 0
    for c in range(NCORES):
        m = {}
        qorder = np.concatenate([clusters[c][t] for t in range(NT)])
        order[pos:pos + NT * P] = qorder
        pos += NT * P

        idx = np.empty((P, NT), np.int32)
        wts = np.stack([(1.0 - fy[qorder]), fy[qorder],
                        (1.0 - fx[qorder]) / 16.0, fx[qorder] / 16.0],
                       axis=1).astype(np.float32)
        wtsP = wts.reshape(NT, P, 4).transpose(1, 0, 2).reshape(P, NT * 4)

        slabs = []
        for t in range(NT):
            qs = clusters[c][t]
            px, py = shapes[t]
            g, j = divmod(t, 2)
            x0 = int(jx[qs].min()) - R
            y0 = int(jy[qs].min()) - R
            # zero-padded [C, pads[t]] patch slab (x-major, y minor)
            slab = np.zeros((C, pads[t]), np.float32)
            sl = slab[:, :px * py].reshape(C, px, py)
            xs0, xs1 = max(x0, 0), min(x0 + px, W)
            ys0, ys1 = max(y0, 0), min(y0 + py, H)
            if xs1 > xs0 and ys1 > ys0:
                sl[:, xs0 - x0:xs1 - x0, ys0 - y0:ys1 - y0] = \
                    f2[:, ys0:ys1, xs0:xs1].transpose(0, 2, 1)
            sb = slab.reshape(2, P, pads[t]).astype(bf16)
            slabs.append(np.ascontiguousarray(
                np.concatenate([sb[0], sb[1]], axis=1)).view(np.uint8))
            sg = pads[2 * g] + pads[2 * g + 1]
            base = pads[2 * g] if j else 0
            rel = (jx[qs] - R - x0) * py + (jy[qs] - R - y0)
            idx[:, t] = (np.arange(P) * sg + base + rel).astype(np.int32)

        f1b = f1[:, qorder].reshape(2, P, NT * P).astype(bf16)
        for g in range(NG):
            f1g = np.ascontiguousarray(np.concatenate(
                [f1b[0, :, g * 256:(g + 1) * 256],
                 f1b[1, :, g * 256:(g + 1) * 256]], axis=1)).view(np.uint8)
            parts = [f1g, slabs[2 * g], slabs[2 * g + 1]]
            if g == 0:
                parts = [idx.view(np.uint8),
                         np.ascontiguousarray(wtsP).view(np.uint8)] + parts
            m[f"in{g}"] = np.ascontiguousarray(np.concatenate(parts, axis=1))
        in_maps.append(m)
    return in_maps, order, shapes


def assemble_output(results, order):
    # device emits [P, NT*81] partition-major; restore (tile, p) query order
    rows = np.concatenate(
        [results[c]["out"].reshape(P, NT, K * K).transpose(1, 0, 2)
         .reshape(NT * P, K * K) for c in range(NCORES)],
        axis=0)
    # device blend emits [dx, dy]-major, matching the reference's 81-axis
    # (delta[..., 0] is added to x and varies along the first grid axis)
    full = np.empty((K * K, NQ), np.float32)
    full[:, order] = rows.T.astype(np.float32)
    return full.reshape(1, K * K, H, W)


# --------------------------------------------------------------------------
# device program
# --------------------------------------------------------------------------

IDX_BYTES = NT * 4          # [P, NT] i32
WTS_BYTES = NT * 4 * 4      # [P, NT*4] f32
F1G_BYTES = 2 * 256 * 2     # [P, 2*256] bf16 per group


def _body(tc, nc, aps, scr, shapes, pools, parity=0):
    const, corr_pool, psum_pool, small = pools
    e0, e1 = (nc.sync, nc.scalar) if parity == 0 else (nc.scalar, nc.sync)
    pads = [BANK * ((px * py + BANK - 1) // BANK) for px, py in shapes]
    sg_sizes = [pads[2 * g] + pads[2 * g + 1] for g in range(NG)]
    wins = [(PK - 1) * py + PK for _, py in shapes]
    wmaxs = [max(wins[2 * g], wins[2 * g + 1]) for g in range(NG)]

    packs = []
    for g in range(NG):
        hdr = (IDX_BYTES + WTS_BYTES) if g == 0 else 0
        nbytes = hdr + F1G_BYTES + 2 * (pads[2 * g] + pads[2 * g + 1]) * 2
        pk = const.tile([P, nbytes], U8, tag=f"pack{g}")
        [e0, e1, e0][g].dma_start(pk[:], aps[f"in{g}"])
        packs.append(pk)

    idxb = packs[0][:, 0:IDX_BYTES].bitcast(I32)                   # [P, NT]
    wtsb = packs[0][:, IDX_BYTES:IDX_BYTES + WTS_BYTES].bitcast(F32)

    otb = const.tile([P, NT * K * K], F32, tag="otb")

    for g in range(NG):
        hdr = (IDX_BYTES + WTS_BYTES) if g == 0 else 0
        f1g = packs[g][:, hdr:hdr + F1G_BYTES].bitcast(BF16)       # [P, 512]
        sg = sg_sizes[g]
        wmax = wmaxs[g]

        ps = psum_pool.tile([P, sg], F32, space="PSUM", tag="ps")
        for j in range(2):
            t = 2 * g + j
            pad = pads[t]
            off = pads[2 * g] if j else 0
            f2v = packs[g][:, hdr + F1G_BYTES + off * 4:
                           hdr + F1G_BYTES + (off + pad) * 4].bitcast(BF16)
            for ci in range(pad // BANK):
                for k in range(2):
                    lhsT = f1g[:, k * 256 + j * P: k * 256 + (j + 1) * P]
                    rhs = f2v[:, k * pad + ci * BANK:
                              k * pad + ci * BANK + BANK]
                    nc.tensor.matmul(
                        ps[:, off + ci * BANK: off + (ci + 1) * BANK],
                        lhsT=lhsT, rhs=rhs, start=(k == 0), stop=(k == 1))

        corr_g = corr_pool.tile([P, max(sg_sizes)], F16, tag="corr")
        nc.vector.tensor_copy(corr_g[:, 0:sg], ps[:])

        dst = scr[g].ap()[0:P * sg].rearrange("(p f) -> p f", p=P)
        [e1, e0, e1][g].dma_start(dst, corr_g[:, 0:sg])

        src = scr[g].ap().rearrange("(n o) -> n o", o=1)
        pts = []
        for j in range(2):
            t = 2 * g + j
            # multi-offset indirect DMA is broken on HW: one gather per tile
            pt = small.tile([P, PK * shapes[t][1]], F16, tag=f"pt{j}")
            nc.gpsimd.indirect_dma_start(
                out=pt[:, 0:wins[t]], out_offset=None, in_=src,
                in_offset=bass.IndirectOffsetOnAxis(
                    ap=idxb[:, t:t + 1], axis=0))
            pts.append(pt)

        for j in range(2):
            t = 2 * g + j
            py = shapes[t][1]
            ptv = pts[j][:].rearrange("p (b r) -> p b r", r=py)[:, :, 0:PK]

            t1 = small.tile([P, PK * K], F16, tag="t1")
            t13 = t1[:].rearrange("p (a b) -> p a b", b=K)
            nc.vector.tensor_scalar_mul(
                t13, ptv[:, :, 1:PK], wtsb[:, 4 * t + 1: 4 * t + 2])
            cm = small.tile([P, PK * K], F16, tag="cm")
            cm3 = cm[:].rearrange("p (a b) -> p a b", b=K)
            nc.vector.scalar_tensor_tensor(
                cm3, ptv[:, :, 0:K], wtsb[:, 4 * t: 4 * t + 1], t13,
                op0=mybir.AluOpType.mult, op1=mybir.AluOpType.add)

            t2 = small.tile([P, K * K], F16, tag="t2")
            t23 = t2[:].rearrange("p (a b) -> p a b", b=K)
            nc.vector.tensor_scalar_mul(
                t23, cm3[:, 1:PK, :], wtsb[:, 4 * t + 3: 4 * t + 4])
            ot3 = otb[:, t * K * K:(t + 1) * K * K] \
                .rearrange("p (a b) -> p a b", b=K)
            nc.vector.scalar_tensor_tensor(
                ot3, cm3[:, 0:K, :], wtsb[:, 4 * t + 2: 4 * t + 3], t23,
                op0=mybir.AluOpType.mult, op1=mybir.AluOpType.add)

    # out is partition-major [P, NT*81]; the host transposes to query order
    e0.dma_start(aps["out"], otb[:])


def build_program(shapes, rep=1):
    """rep>1 wraps a double body in a For_i(rep//2) loop (for timing)."""
    nc = bacc.Bacc("TRN2", target_bir_lowering=False, debug=False,
                   num_devices=NCORES)
    pads = [BANK * ((px * py + BANK - 1) // BANK) for px, py in shapes]
    aps = {}
    for g in range(NG):
        hdr = (IDX_BYTES + WTS_BYTES) if g == 0 else 0
        nbytes = hdr + F1G_BYTES + 2 * (pads[2 * g] + pads[2 * g + 1]) * 2
        aps[f"in{g}"] = nc.dram_tensor(f"in{g}", [P, nbytes], U8,
                                       kind="ExternalInput").ap()
    aps["out"] = nc.dram_tensor("out", [P, NT * K * K], F32,
                                kind="ExternalOutput").ap()
    psum_bufs = max(1, min(4, 8 * BANK // max(
        pads[2 * g] + pads[2 * g + 1] for g in range(NG))))
    unroll = min(2, rep)
    scr = [[nc.dram_tensor(f"scr{b}_{g}",
                           [P * (pads[2 * g] + pads[2 * g + 1])], F16)
            for g in range(NG)] for b in range(unroll)]

    with tile.TileContext(nc) as tc:
        def mk_pools(ctx):
            return (ctx.enter_context(tc.tile_pool(name="const", bufs=3)),
                    ctx.enter_context(tc.tile_pool(name="corr", bufs=3)),
                    ctx.enter_context(
                        tc.tile_pool(name="ps", bufs=psum_bufs,
                                     space="PSUM")),
                    ctx.enter_context(tc.tile_pool(name="small", bufs=3)))

        import contextlib
        with contextlib.ExitStack() as ctx:
            pools = mk_pools(ctx)
            if rep == 1:
                _body(tc, nc, aps, scr[0], shapes, pools)
            else:
                tail = rep % unroll
                with tc.For_i(0, rep // unroll, staggered_reset=True):
                    for b in range(unroll):
                        _body(tc, nc, aps, scr[b], shapes, pools, b % 2)
                for b in range(tail):
                    _body(tc, nc, aps, scr[b], shapes, pools, b % 2)
    nc.compile()
    return nc


_PROGRAMS = {}


def kernel(fmap1, fmap2, coords, radius):
    assert int(radius) == R, f"kernel hardcodes radius=4, got {radius}"
    in_maps, order, shapes = host_preprocess(fmap1, fmap2, coords)
    nc = _PROGRAMS.get(shapes)
    if nc is None:
        nc = _PROGRAMS[shapes] = build_program(shapes)
    last_err = None
    for _ in range(3):  # the remote compile hook occasionally flakes
        try:
            res = bass_utils.run_bass_kernel_spmd(
                nc, in_maps, core_ids=list(range(NCORES)))
            return assemble_output(res.results, order)
        except Exception as e:  # noqa: BLE001
            last_err = e
    raise last_err
